# revision 18
# baseline (speedup 1.0000x reference)
"""Trainium2 Bass kernel for nn_EmotionClassifier (ResNet18 + 2-layer GAT + FC).

Data-parallel across 8 NeuronCores: 4 images/graphs per core, params replicated.
Host-side numpy does layout prep only (BN folding, im2col, edge padding); all
model compute runs on device via a Bass/Tile program executed with
concourse.bass_utils.run_bass_kernel_spmd.
"""
import os
import numpy as np
from contextlib import ExitStack

import concourse.bass as bass
import concourse.tile as tile
import concourse.mybir as mybir
from concourse.alu_op_type import AluOpType
from concourse.bass_utils import run_bass_kernel_spmd

F32 = mybir.dt.float32
F32R = mybir.dt.float32r
AF = mybir.ActivationFunctionType

B, NNODE, NEDGE, HEADS, DIM = 32, 68, 256, 4, 64
GAT_OUT = HEADS * DIM
NCORES = 8
PER = B // NCORES            # 4 images per core
EP = NEDGE + NNODE           # 324 edges incl self loops
ECH = 3                      # edge chunks of 128 (padded to 384)
NCLS = 7

RES_DT = F32 if os.environ.get("KERNEL_FP32") else F32R


# ---------------------------------------------------------------- host prep
def _fold_bn(w, bn):
    s = np.asarray(bn["g"], np.float64) / np.sqrt(np.asarray(bn["v"], np.float64) + 1e-5)
    t = np.asarray(bn["b"], np.float64) - np.asarray(bn["m"], np.float64) * s
    return (np.asarray(w, np.float64) * s[:, None, None, None]).astype(np.float32), t.astype(np.float32)


def _prep_conv(w, bn):
    """-> lhsT [npos, Cin, Cout] fp32, bias [Cout]"""
    wf, t = _fold_bn(w, bn)
    lhsT = np.transpose(wf, (2, 3, 1, 0)).reshape(-1, wf.shape[1], wf.shape[0])
    return np.ascontiguousarray(lhsT), t


class _Packer:
    """Packs per-(pos,kc,mc) [<=128, 128] weight blocks into a [128, cols]
    blob; records offsets for device-side slicing."""

    def __init__(self):
        self.cols = []
        self.ofs = {}
        self.n = 0

    def add(self, name, lhsT):
        npos, cin, cout = lhsT.shape
        KC = (cin + 127) // 128
        MC = (cout + 127) // 128
        self.ofs[name] = (self.n, npos, KC, MC, cin, cout)
        for pos in range(npos):
            for kc in range(KC):
                for mc in range(MC):
                    blk = np.zeros((128, 128), np.float32)
                    ks = min(128, cin - kc * 128)
                    ms = min(128, cout - mc * 128)
                    blk[:ks, :ms] = lhsT[pos, kc * 128 : kc * 128 + ks,
                                         mc * 128 : mc * 128 + ms]
                    self.cols.append(blk)
                    self.n += 128

    def blob(self):
        return np.ascontiguousarray(np.concatenate(self.cols, axis=1))


def _im2col(img):
    """img [224,224] -> [49, 12544] (7x7 s2 pad 3)."""
    pad = np.zeros((230, 230), np.float32)
    pad[3:227, 3:227] = img
    out = np.empty((49, 112 * 112), np.float32)
    for ky in range(7):
        for kx in range(7):
            out[ky * 7 + kx] = pad[ky : ky + 223 : 2, kx : kx + 223 : 2].reshape(-1)
    return out


def _host_prep(image, landmarks, edge_index, params):
    image = np.asarray(image, np.float32)
    landmarks = np.asarray(landmarks, np.float32)
    edge_index = np.asarray(edge_index)
    rp = params["resnet"]

    packs = {}
    biases = []
    bias_ofs = {}

    def add_bias(name, t):
        cout = t.shape[0]
        MC = (cout + 127) // 128
        bias_ofs[name] = len(biases)
        for mc in range(MC):
            col = np.zeros((128, 1), np.float32)
            ms = min(128, cout - mc * 128)
            col[:ms, 0] = t[mc * 128 : mc * 128 + ms]
            biases.append(col)

    pk = _Packer()
    w, t = _prep_conv(rp["conv1"], rp["bn1"])   # [49, 1, 64]
    pk.add("conv1", np.ascontiguousarray(w.reshape(1, 49, 64)))
    add_bias("conv1", t)
    packs["wbc1"] = pk

    pk = _Packer()
    for j, bp in enumerate(rp["layer1"]):
        for cw, cb in [("conv1", "bn1"), ("conv2", "bn2")]:
            w, t = _prep_conv(bp[cw], bp[cb])
            pk.add(f"l1b{j}{cw}", w)
            add_bias(f"l1b{j}{cw}", t)
    packs["wb1"] = pk

    for lname, key in [("layer2", "l2"), ("layer3", "l3")]:
        pk = _Packer()
        for j, bp in enumerate(rp[lname]):
            for cw, cb in [("conv1", "bn1"), ("conv2", "bn2")]:
                w, t = _prep_conv(bp[cw], bp[cb])
                pk.add(f"{key}b{j}{cw}", w)
                add_bias(f"{key}b{j}{cw}", t)
            if j == 0:
                w, t = _prep_conv(bp["down_conv"], bp["down_bn"])
                pk.add(f"{key}down", w)
                add_bias(f"{key}down", t)
        packs[f"wb{key[1]}"] = pk

    l4 = rp["layer4"]
    for tag, j, cw, cb in [("wb4a", 0, "conv1", "bn1"), ("wb4b", 0, "conv2", "bn2"),
                           ("wb4c", 1, "conv1", "bn1"), ("wb4d", 1, "conv2", "bn2")]:
        pk = _Packer()
        w, t = _prep_conv(l4[j][cw], l4[j][cb])
        pk.add(f"l4b{j}{cw}", w)
        add_bias(f"l4b{j}{cw}", t)
        if tag == "wb4a":
            w, t = _prep_conv(l4[0]["down_conv"], l4[0]["down_bn"])
            pk.add("l4down", w)
            add_bias("l4down", t)
        packs[tag] = pk

    bias_blob = np.ascontiguousarray(np.concatenate(biases, axis=1))  # [128, nb]

    # GAT + FC consts (fp32)
    g1, g2 = params["gat1"], params["gat2"]
    W1 = np.asarray(g1["w"], np.float32)            # [2, 256]
    W2 = np.asarray(g2["w"], np.float32)            # [256, 256]
    W2p = np.zeros((128, 2, 256), np.float32)
    W2p[:, 0] = W2[:128]
    W2p[:, 1] = W2[128:]

    def att_mat(g):
        # column j = sd*4 + hd ; sd: 0 = dst, 1 = src
        a_s = np.asarray(g["att_src"], np.float32).reshape(HEADS, DIM)
        a_d = np.asarray(g["att_dst"], np.float32).reshape(HEADS, DIM)
        m = np.zeros((128, 2, 8), np.float32)
        for hd in range(HEADS):
            ch, lo = divmod(hd * DIM, 128)
            m[lo : lo + DIM, ch, hd] = a_d[hd]
            m[lo : lo + DIM, ch, 4 + hd] = a_s[hd]
        return m

    ATT1, ATT2 = att_mat(g1), att_mat(g2)
    fcw = np.asarray(params["fc_out"]["w"], np.float32).copy()  # [768, 7]
    fcw[GAT_OUT:] /= 49.0     # fold avgpool mean (device computes sums)
    fcw_p = np.zeros((128, 6, NCLS), np.float32)
    for j in range(6):
        fcw_p[:, j] = fcw[j * 128 : (j + 1) * 128]
    ident = np.eye(128, dtype=np.float32)

    loops = np.arange(NNODE)
    shared = {
        "bias": bias_blob, "w1g": W1,
        "w2g": np.ascontiguousarray(W2p.reshape(128, 512)),
        "att1": np.ascontiguousarray(ATT1.reshape(128, 16)),
        "att2": np.ascontiguousarray(ATT2.reshape(128, 16)),
        "fcw": np.ascontiguousarray(fcw_p.reshape(128, 6 * NCLS)),
        "ident": ident,
        "zeros": np.zeros((128, 512), np.float32),
    }
    for name, p in packs.items():
        shared[name] = p.blob()

    in_maps = []
    for c in range(NCORES):
        lo = c * PER
        ic = np.stack([_im2col(image[b, 0]) for b in range(lo, lo + PER)], 1)
        xT = np.ascontiguousarray(
            np.stack([landmarks[lo + s].T for s in range(PER)], 1))  # [2, PER, 68]
        esrc = np.full((128, ECH, PER), 127.0, np.float32)
        edst = np.full((128, ECH, PER), 127.0, np.float32)
        for s in range(PER):
            b = lo + s
            src = np.concatenate([edge_index[b, 0], loops]).astype(np.float32)
            dst = np.concatenate([edge_index[b, 1], loops]).astype(np.float32)
            for ch in range(ECH):
                a, bnd = ch * 128, min((ch + 1) * 128, EP)
                if bnd > a:
                    esrc[: bnd - a, ch, s] = src[a:bnd]
                    edst[: bnd - a, ch, s] = dst[a:bnd]
        m = dict(shared)
        m["ic"] = np.ascontiguousarray(ic.reshape(49, PER * 12544))
        m["xt"] = np.ascontiguousarray(xT.reshape(2, PER * NNODE))
        m["esrc"] = np.ascontiguousarray(esrc.reshape(128, ECH * PER))
        m["edst"] = np.ascontiguousarray(edst.reshape(128, ECH * PER))
        in_maps.append(m)

    meta = {"packs": {k: v.ofs for k, v in packs.items()},
            "bias_ofs": bias_ofs,
            "blob_cols": {k: v.blob().shape[1] for k, v in packs.items()},
            "nb": bias_blob.shape[1]}
    return in_maps, meta


# ------------------------------------------------- walrus single-wait fixup
_nopctr = [0]


def _fixup_sync_waits(nc):
    """This walrus build allows one sync-wait per instruction; hoist extras
    onto preceding NoOps on the same engine."""
    for fn in nc.m.functions:
        for bb in fn.blocks:
            out = []
            changed = False
            for inst in bb.instructions:
                si = inst.sync_info
                waits = list(si.on_wait) if si and si.on_wait else []
                if len(waits) > 1:
                    changed = True
                    for w in waits[:-1]:
                        _nopctr[0] += 1
                        nop = mybir.InstNoOp(name=f"waitnop_{_nopctr[0]}", ins=[], outs=[])
                        nop.engine = inst.engine
                        nop.sync_info = mybir.SyncInfo(on_wait=[w], on_update=[])
                        out.append(nop)
                    si.on_wait[:] = [waits[-1]]
                out.append(inst)
            if changed:
                bb.instructions = out


class _TileCtx(tile.TileContext):
    def __exit__(self, *args):
        r = super().__exit__(*args)
        _fixup_sync_waits(self.nc)
        return r


# ---------------------------------------------------------------- program
def _build_program(meta):
    nc = bass.Bass("TRN2", target_bir_lowering=False, debug=False)
    pofs = meta["packs"]
    bofs = meta["bias_ofs"]
    bcols = meta["blob_cols"]

    dr = {}
    dr["ic"] = nc.dram_tensor("ic", [49, PER * 12544], RES_DT, kind="ExternalInput").ap()
    dr["xt"] = nc.dram_tensor("xt", [2, PER * NNODE], F32, kind="ExternalInput").ap()
    dr["esrc"] = nc.dram_tensor("esrc", [128, ECH * PER], F32, kind="ExternalInput").ap()
    dr["edst"] = nc.dram_tensor("edst", [128, ECH * PER], F32, kind="ExternalInput").ap()
    dr["bias"] = nc.dram_tensor("bias", [128, meta["nb"]], F32, kind="ExternalInput").ap()
    dr["w1g"] = nc.dram_tensor("w1g", [2, 256], F32, kind="ExternalInput").ap()
    dr["w2g"] = nc.dram_tensor("w2g", [128, 512], F32, kind="ExternalInput").ap()
    dr["att1"] = nc.dram_tensor("att1", [128, 16], F32, kind="ExternalInput").ap()
    dr["att2"] = nc.dram_tensor("att2", [128, 16], F32, kind="ExternalInput").ap()
    dr["fcw"] = nc.dram_tensor("fcw", [128, 6 * NCLS], F32, kind="ExternalInput").ap()
    dr["ident"] = nc.dram_tensor("ident", [128, 128], F32, kind="ExternalInput").ap()
    dr["zeros"] = nc.dram_tensor("zeros", [128, 512], RES_DT, kind="ExternalInput").ap()
    for k in ["wbc1", "wb1", "wb2", "wb3", "wb4a", "wb4b", "wb4c", "wb4d"]:
        dr[k] = nc.dram_tensor(k, [128, bcols[k]], RES_DT, kind="ExternalInput").ap()
    out = nc.dram_tensor("out", [NCLS, PER], F32, kind="ExternalOutput").ap()

    dbg = {}
    if os.environ.get("KERNEL_DEBUG"):
        dbg["l1in"] = nc.dram_tensor("dbg_l1in", [64, PER * 58 * 58], F32, kind="ExternalOutput").ap()
        dbg["l2out"] = nc.dram_tensor("dbg_l2out", [128, PER * 900], F32, kind="ExternalOutput").ap()
        dbg["g1"] = nc.dram_tensor("dbg_g1", [NNODE, 256], F32, kind="ExternalOutput").ap()
        dbg["zt"] = nc.dram_tensor("dbg_zt", [128, 6 * PER], F32, kind="ExternalOutput").ap()

    with _TileCtx(nc, pool_alloc_mode="queue") as tc:
        with ExitStack() as top:
            cst = top.enter_context(tc.tile_pool(name="cst", bufs=1))

            identt = cst.tile([128, 128], F32)
            nc.sync.dma_start(identt[:], dr["ident"][:])
            biast = cst.tile([128, meta["nb"]], F32)
            nc.sync.dma_start(biast[:], dr["bias"][:])
            zt = cst.tile([128, 6 * PER], F32)   # FC rhs cols [128, (j, s)]
            ztv = zt[:].rearrange("p (j s) -> p j s", s=PER)
            zrow = cst.tile([128, 512], RES_DT)
            nc.sync.dma_start(zrow[:], dr["zeros"][:])
            ones_col = cst.tile([NNODE, 1], F32)
            nc.vector.memset(ones_col[:], 1.0)
            mean_col = cst.tile([NNODE, 1], F32)
            nc.vector.memset(mean_col[:], 1.0 / NNODE)

            def bias_ap(name, mc, ms=128):
                return biast[0:ms, bofs[name] + mc : bofs[name] + mc + 1]

            _gat_phase(nc, tc, dr, ztv, identt, ones_col, mean_col, dbg)
            _resnet_phase(nc, tc, dr, pofs, bias_ap, ztv, zrow, dbg)

            # -------- FC
            with tc.tile_pool(name="fcp", bufs=1) as fcp, \
                 tc.tile_pool(name="fcps", bufs=1, space="PSUM") as fcps:
                fcw_t = fcp.tile([128, 6 * NCLS], F32)
                nc.sync.dma_start(fcw_t[:], dr["fcw"][:])
                fv = fcw_t[:].rearrange("p (j c) -> p j c", c=NCLS)
                pfc = fcps.tile([NCLS, PER], F32)
                for j in range(6):
                    nc.tensor.matmul(pfc[:], fv[:, j, :], ztv[:, j, :],
                                     start=(j == 0), stop=(j == 5))
                out_sb = fcp.tile([NCLS, PER], F32)
                nc.vector.tensor_copy(out_sb[:], pfc[:])
                nc.sync.dma_start(out[:], out_sb[:])
                if "zt" in dbg:
                    nc.sync.dma_start(dbg["zt"][:], zt[:])
    return nc


# ---------------------------------------------------------------- GAT
def _gat_phase(nc, tc, dr, ztv, identt, ones_col, mean_col, dbg):
    with ExitStack() as es:
        gc = es.enter_context(tc.tile_pool(name="gatc", bufs=1))
        gp = es.enter_context(tc.tile_pool(name="gatw", bufs=2))
        ps = es.enter_context(tc.tile_pool(name="gatp", bufs=3, space="PSUM"))

        def pg():
            return ps.tile([128, 512], F32, tag="pg", name="pg")

        xt = gc.tile([2, PER * NNODE], F32)
        nc.sync.dma_start(xt[:], dr["xt"][:])
        esrc = gc.tile([128, ECH * PER], F32)
        nc.sync.dma_start(esrc[:], dr["esrc"][:])
        esv = esrc[:].rearrange("p (c s) -> p c s", s=PER)
        edst = gc.tile([128, ECH * PER], F32)
        nc.sync.dma_start(edst[:], dr["edst"][:])
        edv = edst[:].rearrange("p (c s) -> p c s", s=PER)
        w1 = gc.tile([2, 256], F32)
        nc.sync.dma_start(w1[:], dr["w1g"][:])
        w2 = gc.tile([128, 512], F32)
        nc.sync.dma_start(w2[:], dr["w2g"][:])
        w2v = w2[:].rearrange("p (k c) -> p k c", k=2)
        att1 = gc.tile([128, 16], F32)
        nc.sync.dma_start(att1[:], dr["att1"][:])
        att2 = gc.tile([128, 16], F32)
        nc.sync.dma_start(att2[:], dr["att2"][:])
        attv = {1: att1[:].rearrange("p (c j) -> p c j", j=8),
                2: att2[:].rearrange("p (c j) -> p c j", j=8)}
        iot = gc.tile([128, NNODE], F32)
        nc.gpsimd.iota(iot[:], pattern=[[1, NNODE]], base=0, channel_multiplier=0,
                       allow_small_or_imprecise_dtypes=True)
        ct_all = gc.tile([NNODE, PER * NNODE], F32)
        ctv = ct_all[:].rearrange("p (s n) -> p s n", s=PER)

        for s in range(PER):
            pc = pg()
            for ch in range(ECH):
                ohs = gp.tile([128, NNODE], F32, tag="ohs")
                nc.vector.tensor_scalar(ohs[:], iot[:], esv[:, ch, s : s + 1], None,
                                        AluOpType.is_equal)
                ohd = gp.tile([128, NNODE], F32, tag="ohd")
                nc.vector.tensor_scalar(ohd[:], iot[:], edv[:, ch, s : s + 1], None,
                                        AluOpType.is_equal)
                nc.tensor.matmul(pc[0:NNODE, 0:NNODE], ohs[:], ohd[:],
                                 start=(ch == 0), stop=(ch == ECH - 1))
            nc.vector.tensor_copy(ctv[:, s, :], pc[0:NNODE, 0:NNODE])

        for s in range(PER):
            xts = xt[:, s * NNODE : (s + 1) * NNODE]       # [2, 68]
            ph = pg()
            nc.tensor.matmul(ph[0:NNODE, 0:256], xts, w1[:], start=True, stop=True)
            h = gp.tile([NNODE, 256], F32, tag="h")
            nc.vector.tensor_copy(h[:], ph[0:NNODE, 0:256])
            hT = gp.tile([128, 2 * NNODE], F32, tag="hT")
            hTv = hT[:].rearrange("p (k n) -> p k n", k=2)
            for ck in range(2):
                pt = pg()
                nc.tensor.matmul(pt[:, 0:NNODE], w1[:, ck * 128 : (ck + 1) * 128],
                                 xts, start=True, stop=True)
                nc.vector.tensor_copy(hTv[:, ck, :], pt[:, 0:NNODE])

            g = None
            for layer in (1, 2):
                pa = pg()
                for ck in range(2):
                    nc.tensor.matmul(pa[0:8, 0:NNODE], attv[layer][:, ck, :],
                                     hTv[:, ck, :], start=(ck == 0), stop=(ck == 1))
                pa_sb = gp.tile([8, NNODE], F32, tag="pa_sb")
                nc.vector.tensor_copy(pa_sb[:], pa[0:8, 0:NNODE])
                # lhs_all: p0 = ones, p1 = a_src rows; rhs_all: p0 = a_dst, p1 = ones
                lhs_all = gp.tile([2, HEADS * NNODE], F32, tag="lhs_all")
                lav = lhs_all[:].rearrange("p (h n) -> p h n", h=HEADS)
                nc.vector.memset(lhs_all[:], 1.0)
                nc.sync.dma_start(lav[1:2, :, :], pa_sb[4:8, :])
                rhs_all = gp.tile([2, HEADS * NNODE], F32, tag="rhs_all")
                rav = rhs_all[:].rearrange("p (h n) -> p h n", h=HEADS)
                nc.vector.memset(rhs_all[:], 1.0)
                nc.sync.dma_start(rav[0:1, :, :], pa_sb[0:4, :])

                g = gp.tile([NNODE, 256], F32, tag=f"g{layer}")
                for hd in range(HEADS):
                    ps_ = pg()
                    nc.tensor.matmul(ps_[0:NNODE, 0:NNODE], lav[:, hd, :],
                                     rav[:, hd, :], start=True, stop=True)
                    s_sb = gp.tile([NNODE, NNODE], F32, tag="s_sb")
                    nc.scalar.activation(s_sb[:], ps_[0:NNODE, 0:NNODE], AF.Identity)
                    e_sb = gp.tile([NNODE, NNODE], F32, tag="e_sb")
                    nc.vector.scalar_tensor_tensor(e_sb[:], s_sb[:], 0.2, s_sb[:],
                                                   AluOpType.mult, AluOpType.max)
                    x_sb = gp.tile([NNODE, NNODE], F32, tag="x_sb")
                    nc.scalar.activation(x_sb[:], e_sb[:], AF.Exp)
                    p_sb = gp.tile([NNODE, NNODE], F32, tag="p_sb")
                    nc.vector.tensor_mul(p_sb[:], x_sb[:], ctv[:, s, :])
                    pn = pg()
                    nc.tensor.matmul(pn[0:NNODE, 0:DIM], p_sb[:],
                                     h[:, hd * DIM : (hd + 1) * DIM],
                                     start=True, stop=True, skip_group_check=True)
                    nc.tensor.matmul(pn[0:NNODE, DIM : DIM + 1], p_sb[:], ones_col[:],
                                     start=True, stop=True, skip_group_check=True)
                    rcp = gp.tile([NNODE, 1], F32, tag="rcp")
                    nc.vector.reciprocal(rcp[:], pn[0:NNODE, DIM : DIM + 1])
                    nc.vector.tensor_scalar(g[:, hd * DIM : (hd + 1) * DIM],
                                            pn[0:NNODE, 0:DIM], rcp[:], 0.0,
                                            AluOpType.mult, AluOpType.max)
                if layer == 1:
                    gT = gp.tile([128, 2 * NNODE], F32, tag="gT")
                    gTv = gT[:].rearrange("p (k n) -> p k n", k=2)
                    for ck in range(2):
                        pt = pg()
                        nc.tensor.transpose(pt[:, 0:NNODE],
                                            g[:, ck * 128 : (ck + 1) * 128],
                                            identt[0:NNODE, 0:NNODE])
                        nc.vector.tensor_copy(gTv[:, ck, :], pt[:, 0:NNODE])
                    ph2 = pg()
                    for ck in range(2):
                        nc.tensor.matmul(ph2[0:NNODE, 0:256], gTv[:, ck, :],
                                         w2v[:, ck, :], start=(ck == 0), stop=(ck == 1))
                    h = gp.tile([NNODE, 256], F32, tag="h")
                    nc.vector.tensor_copy(h[:], ph2[0:NNODE, 0:256])
                    hT = gp.tile([128, 2 * NNODE], F32, tag="hT")
                    hTv = hT[:].rearrange("p (k n) -> p k n", k=2)
                    for mk in range(2):
                        pt2 = pg()
                        for ck in range(2):
                            nc.tensor.matmul(pt2[:, 0:NNODE],
                                             w2v[:, ck, mk * 128 : (mk + 1) * 128],
                                             gTv[:, ck, :], start=(ck == 0), stop=(ck == 1))
                        nc.vector.tensor_copy(hTv[:, mk, :], pt2[:, 0:NNODE])

            if "g1" in dbg and s == 0:
                nc.sync.dma_start(dbg["g1"][:], g[:])
            for ck in range(2):
                pm = pg()
                nc.tensor.matmul(pm[:, 0:1], g[:, ck * 128 : (ck + 1) * 128],
                                 mean_col[:], start=True, stop=True)
                nc.vector.tensor_copy(ztv[:, ck, s : s + 1], pm[:, 0:1])


# ---------------------------------------------------------------- ResNet
def _resnet_phase(nc, tc, dr, pofs, bias_ap, ztv, zrow, dbg):
    L1, L2, L3, L4 = 58 * 58, 30 * 30, 16 * 16, 9 * 9

    def wmeta(pack, name):
        return pofs[pack][name]

    def full_wprov(wt, pack, name):
        base0 = wmeta(pack, name)[0]

        def f(pos, kc, mc, ks, ms):
            _, npos, KC, MC, cin, cout = wmeta(pack, name)
            col = base0 + ((pos * KC + kc) * MC + mc) * 128
            return wt[0:ks, col : col + ms]
        return f

    def halo_zero(buf_v, Hp):
        g = buf_v.rearrange("p m s (r c) -> p m s r c", r=Hp)
        P, m, si = g.shape[0], g.shape[1], g.shape[2]
        zv = zrow[0:P, 0 : m * si * Hp].rearrange("p (m s c) -> p m s c", m=m, s=si)
        nc.vector.tensor_copy(g[:, :, :, 0, :], zv)
        nc.vector.tensor_copy(g[:, :, :, Hp - 1, :], zv)
        nc.vector.tensor_copy(g[:, :, :, :, 0], zv)
        nc.vector.tensor_copy(g[:, :, :, :, Hp - 1], zv)

    def conv_s1(psum, tmp_pool, x_v, y_v, pack, name, Hp, relu, wprov,
                residual=None):
        """x_v,y_v: [p, KC/MC, S, L] views. 3x3 s1 flat-halo conv."""
        _, npos, KC, MC, cin, cout = wmeta(pack, name)
        Wp = Hp
        nimg = x_v.shape[2]
        Ltot = nimg * Hp * Wp
        S = Wp + 1
        ms = min(128, cout)
        ks = min(128, cin)
        xf = x_v.rearrange("p k s l -> p k (s l)")
        yf = y_v.rearrange("p m s l -> p m (s l)")
        rf = residual.rearrange("p m s l -> p m (s l)") if residual is not None else None
        for c0 in range(S, Ltot - S, 512):
            n = min(512, Ltot - S - c0)
            for mc in range(MC):
                pc = psum.tile([128, 512], F32, tag="pconv", name="pconv")[0:ms, 0:n]
                first = True
                for dy in range(3):
                    for dx in range(3):
                        off = (dy - 1) * Wp + (dx - 1)
                        for kc in range(KC):
                            nc.tensor.matmul(
                                pc, wprov(dy * 3 + dx, kc, mc, ks, ms),
                                xf[:, kc, c0 + off : c0 + off + n],
                                start=first,
                                stop=(dy == 2 and dx == 2 and kc == KC - 1))
                            first = False
                dst = yf[0:ms, mc, c0 : c0 + n]
                if residual is not None:
                    t = tmp_pool.tile([128, 512], RES_DT, tag="ctmp", name="ctmp")[0:ms, 0:n]
                    nc.vector.scalar_tensor_tensor(t, pc, bias_ap(name, mc, ms),
                                                   rf[0:ms, mc, c0 : c0 + n],
                                                   AluOpType.add, AluOpType.add)
                    if relu:
                        nc.vector.tensor_scalar(dst, t, 0.0, None, AluOpType.max)
                    else:
                        nc.vector.tensor_copy(dst, t)
                else:
                    nc.scalar.activation(dst, pc, AF.Relu if relu else AF.Identity,
                                         bias=bias_ap(name, mc, ms))
        halo_zero(y_v, Hp)

    def conv_s2(psum, x_v, y_v, pack, name, Hp_in, Hp_out, chunks, relu, wprov,
                do_halo=True):
        """x_v [p, KC, Sx, Lin]; y_v [p, MC, Sy, Lout]; chunks = (sx, sy, ns, r0, nr)."""
        _, npos, KC, MC, cin, cout = wmeta(pack, name)
        k = 3 if npos == 9 else 1
        Ho = Hp_out - 2
        Hoc = Ho + (Ho % 2)   # fp32r: innermost moving count must be even
        ms = min(128, cout)
        ks = min(128, cin)
        xg = x_v.rearrange("p k s (r c) -> p k s r c", r=Hp_in)
        yg = y_v.rearrange("p m s (r c) -> p m s r c", r=Hp_out)
        for (sx, sy, ns, r0, nr) in chunks:
            for mc in range(MC):
                pc = psum.tile([128, 512], F32, tag="pconv", name="pconv")
                pcv = pc[0:ms, 0 : ns * nr * Hoc].rearrange(
                    "p (s r c) -> p s r c", s=ns, r=nr)
                first = True
                for dy in range(k):
                    for dx in range(k):
                        for kc in range(KC):
                            if k == 3:
                                v = xg[:, kc, sx : sx + ns,
                                       dy + 2 * r0 : dy + 2 * r0 + 2 * nr - 1 : 2,
                                       dx : dx + 2 * Hoc - 1 : 2]
                            else:
                                v = xg[:, kc, sx : sx + ns,
                                       1 + 2 * r0 : 1 + 2 * r0 + 2 * nr - 1 : 2,
                                       1 : 1 + 2 * Hoc - 1 : 2]
                            nc.tensor.matmul(
                                pcv, wprov(dy * k + dx, kc, mc, ks, ms),
                                v, start=first,
                                stop=(dy == k - 1 and dx == k - 1 and kc == KC - 1))
                            first = False
                dst = yg[0:ms, mc, sy : sy + ns, 1 + r0 : 1 + r0 + nr, 1 : 1 + Ho]
                nc.scalar.activation(dst, pcv[:, :, :, 0:Ho],
                                     AF.Relu if relu else AF.Identity,
                                     bias=bias_ap(name, mc, ms))
        if do_halo:
            halo_zero(y_v, Hp_out)

    with ExitStack() as es0:
        psum = es0.enter_context(tc.tile_pool(name="cpsum", bufs=4, space="PSUM"))
        tmp = es0.enter_context(tc.tile_pool(name="ctmp", bufs=2))
        actS = es0.enter_context(tc.tile_pool(name="actS", bufs=3))

        def sm_tile(MC, L):
            return actS.tile([128, MC * PER * L], RES_DT, tag="sm", name="sm")

        # l2 stride-2 outputs, filled per-image below
        y1_2 = sm_tile(1, L2)
        y1_2v = y1_2[:].rearrange("p (m s l) -> p m s l", m=1, s=PER)
        sc_2 = sm_tile(1, L2)
        sc_2v = sc_2[:].rearrange("p (m s l) -> p m s l", m=1, s=PER)

        with ExitStack() as es1:
            actL1 = es1.enter_context(tc.tile_pool(name="actL1", bufs=3))
            w2ap = es1.enter_context(tc.tile_pool(name="w2ap", bufs=1))
            # l2b0conv1 + l2down weight blocks (prefix of wb2 blob)
            _, npd, KCd, MCd, _, _ = wmeta("wb2", "l2down")
            n2a = wmeta("wb2", "l2down")[0] + npd * KCd * MCd * 128
            wb2a = w2ap.tile([128, n2a], RES_DT)
            nc.sync.dma_start(wb2a[:], dr["wb2"][:, 0:n2a])
            w1p = es1.enter_context(tc.tile_pool(name="w1p", bufs=1))
            wb1 = w1p.tile([128, dr["wb1"].shape[1]], RES_DT)
            nc.sync.dma_start(wb1[:], dr["wb1"][:])
            c1wp = es1.enter_context(tc.tile_pool(name="c1wp", bufs=1))
            wbc1 = c1wp.tile([128, dr["wbc1"].shape[1]], RES_DT)
            nc.sync.dma_start(wbc1[:], dr["wbc1"][:])
            c1sp = es1.enter_context(tc.tile_pool(name="c1sp", bufs=4))
            mpp = es1.enter_context(tc.tile_pool(name="mpp", bufs=1))

            wp_c1 = full_wprov(wbc1, "wbc1", "conv1")
            icv = dr["ic"][:].rearrange("p (s l) -> p s l", s=PER)

            for img in range(PER):
                l1in = actL1.tile([64, L1], RES_DT, tag="l1", name="l1t")
                l1iv = l1in[:].rearrange("p (m s l) -> p m s l", m=1, s=1)
                dst_all = l1in[:].rearrange("p (r c) -> p r c", r=58)
                # conv1 + maxpool in 2 row bands
                for band in range(2):
                    g0 = 0 if band == 0 else 56        # grid row of mp local row 0
                    mp = mpp.tile([64, 58 * 114], RES_DT, tag="mp", name="mp")
                    mpg = mp[:].rearrange("p (r c) -> p r c", r=58)
                    if band == 0:
                        nc.vector.tensor_copy(mpg[:, 0, :], zrow[0:64, 0:114])
                    else:
                        nc.vector.tensor_copy(mpg[:, 57, :], zrow[0:64, 0:114])
                    nc.vector.tensor_copy(mpg[:, :, 0], zrow[0:64, 0:58])
                    nc.vector.tensor_copy(mpg[:, :, 113], zrow[0:64, 0:58])
                    for ck in range(19):
                        r = (0 if band == 0 else 55) + 3 * ck   # conv out row
                        ict = c1sp.tile([49, 336], RES_DT, tag="ic", name="ic")
                        nc.sync.dma_start(ict[:], icv[:, img, r * 112 : (r + 3) * 112])
                        pc = psum.tile([128, 512], F32, tag="pconv", name="pconv")[0:64, 0:336]
                        nc.tensor.matmul(pc, wp_c1(0, 0, 0, 49, 64), ict[:],
                                         start=True, stop=True)
                        lr = r + 1 - g0                          # local grid row
                        nc.scalar.activation(mpg[:, lr : lr + 3, 1:113],
                                             pc.rearrange("p (r c) -> p r c", r=3),
                                             AF.Relu, bias=bias_ap("conv1", 0, 64))
                    # maxpool band -> l1in interior rows
                    ro = 0 if band == 0 else 28              # first out row
                    nr = 28
                    dst = dst_all[:, 1 + ro : 1 + ro + nr, 1:57]
                    first = True
                    for dy in range(3):
                        for dx in range(3):
                            lo = (2 * ro + dy) - g0
                            v = mpg[:, lo : lo + 2 * nr - 1 : 2, dx : dx + 111 : 2]
                            if first:
                                nc.vector.tensor_copy(dst, v)
                                first = False
                            else:
                                nc.vector.tensor_tensor(dst, dst, v, AluOpType.max)
                halo_zero(l1iv, 58)
                if "l1in" in dbg and img == 0:
                    nc.sync.dma_start(dbg["l1in"][:, 0:L1], l1in[:].bitcast(F32))

                # layer1 blocks for this image
                b1y = actL1.tile([64, L1], RES_DT, tag="l1", name="l1t")
                b1yv = b1y[:].rearrange("p (m s l) -> p m s l", m=1, s=1)
                conv_s1(psum, tmp, l1iv, b1yv, "wb1", "l1b0conv1", 58, True,
                        full_wprov(wb1, "wb1", "l1b0conv1"))
                b1o = actL1.tile([64, L1], RES_DT, tag="l1", name="l1t")
                b1ov = b1o[:].rearrange("p (m s l) -> p m s l", m=1, s=1)
                conv_s1(psum, tmp, b1yv, b1ov, "wb1", "l1b0conv2", 58, True,
                        full_wprov(wb1, "wb1", "l1b0conv2"), residual=l1iv)
                b2y = actL1.tile([64, L1], RES_DT, tag="l1", name="l1t")
                b2yv = b2y[:].rearrange("p (m s l) -> p m s l", m=1, s=1)
                conv_s1(psum, tmp, b1ov, b2yv, "wb1", "l1b1conv1", 58, True,
                        full_wprov(wb1, "wb1", "l1b1conv1"))
                l1o = actL1.tile([64, L1], RES_DT, tag="l1", name="l1t")
                l1ov = l1o[:].rearrange("p (m s l) -> p m s l", m=1, s=1)
                conv_s1(psum, tmp, b2yv, l1ov, "wb1", "l1b1conv2", 58, True,
                        full_wprov(wb1, "wb1", "l1b1conv2"), residual=b1ov)

                # fused layer2 stride-2 convs for this image
                ch = [(0, img, 1, 0, 14), (0, img, 1, 14, 14)]
                conv_s2(psum, l1ov, y1_2v, "wb2", "l2b0conv1", 58, 30, ch, True,
                        full_wprov(wb2a, "wb2", "l2b0conv1"), do_halo=False)
                conv_s2(psum, l1ov, sc_2v, "wb2", "l2down", 58, 30, ch, False,
                        full_wprov(wb2a, "wb2", "l2down"), do_halo=False)
        halo_zero(y1_2v, 30)
        halo_zero(sc_2v, 30)

        # -------- layer2 stride-1 blocks
        with ExitStack() as es2:
            w2bp = es2.enter_context(tc.tile_pool(name="w2bp", bufs=1))
            b2base = wmeta("wb2", "l2b0conv2")[0]
            n2b = dr["wb2"].shape[1] - b2base
            wb2b = w2bp.tile([128, n2b], RES_DT)
            nc.sync.dma_start(wb2b[:], dr["wb2"][:, b2base:])

            def w2prov(name):
                f0 = full_wprov(wb2b, "wb2", name)
                def f(pos, kc, mc, ks, ms):
                    ap = f0(pos, kc, mc, ks, ms)
                    return ap.tensor.ap()[0:ks,
                        ap.offset // 4 - b2base : ap.offset // 4 - b2base + ms]                         if False else None
                # simpler: recompute with shifted base
                base0, npos, KC, MC, cin, cout = wmeta("wb2", name)
                def g(pos, kc, mc, ks, ms):
                    col = (base0 - b2base) + ((pos * KC + kc) * MC + mc) * 128
                    return wb2b[0:ks, col : col + ms]
                return g

            b1o2 = sm_tile(1, L2)
            b1o2v = b1o2[:].rearrange("p (m s l) -> p m s l", m=1, s=PER)
            conv_s1(psum, tmp, y1_2v, b1o2v, "wb2", "l2b0conv2", 30, True,
                    w2prov("l2b0conv2"), residual=sc_2v)
            y2_2 = sm_tile(1, L2)
            y2_2v = y2_2[:].rearrange("p (m s l) -> p m s l", m=1, s=PER)
            conv_s1(psum, tmp, b1o2v, y2_2v, "wb2", "l2b1conv1", 30, True,
                    w2prov("l2b1conv1"))
            l2o = sm_tile(1, L2)
            l2ov = l2o[:].rearrange("p (m s l) -> p m s l", m=1, s=PER)
            conv_s1(psum, tmp, y2_2v, l2ov, "wb2", "l2b1conv2", 30, True,
                    w2prov("l2b1conv2"), residual=b1o2v)
            if "l2out" in dbg:
                nc.sync.dma_start(dbg["l2out"][:], l2o[:].bitcast(F32))

        # -------- layer3
        ch3 = [(0, 0, 2, 0, 14), (2, 2, 2, 0, 14)]
        with ExitStack() as es3a:
            w3ap = es3a.enter_context(tc.tile_pool(name="w3ap", bufs=1))
            _, npd, KCd, MCd, _, _ = wmeta("wb3", "l3down")
            n3a = wmeta("wb3", "l3down")[0] + npd * KCd * MCd * 128
            wb3a = w3ap.tile([128, n3a], RES_DT)
            nc.sync.dma_start(wb3a[:], dr["wb3"][:, 0:n3a])
            y1_3 = sm_tile(2, L3)
            y1_3v = y1_3[:].rearrange("p (m s l) -> p m s l", m=2, s=PER)
            conv_s2(psum, l2ov, y1_3v, "wb3", "l3b0conv1", 30, 16, ch3, True,
                    full_wprov(wb3a, "wb3", "l3b0conv1"))
            sc_3 = sm_tile(2, L3)
            sc_3v = sc_3[:].rearrange("p (m s l) -> p m s l", m=2, s=PER)
            conv_s2(psum, l2ov, sc_3v, "wb3", "l3down", 30, 16, ch3, False,
                    full_wprov(wb3a, "wb3", "l3down"))
        with ExitStack() as es3b:
            w3bp = es3b.enter_context(tc.tile_pool(name="w3bp", bufs=1))
            b3base = wmeta("wb3", "l3b0conv2")[0]
            wb3b = w3bp.tile([128, dr["wb3"].shape[1] - b3base], RES_DT)
            nc.sync.dma_start(wb3b[:], dr["wb3"][:, b3base:])

            def w3prov(name):
                base0, npos, KC, MC, cin, cout = wmeta("wb3", name)
                def g(pos, kc, mc, ks, ms):
                    col = (base0 - b3base) + ((pos * KC + kc) * MC + mc) * 128
                    return wb3b[0:ks, col : col + ms]
                return g

            b1o3 = sm_tile(2, L3)
            b1o3v = b1o3[:].rearrange("p (m s l) -> p m s l", m=2, s=PER)
            conv_s1(psum, tmp, y1_3v, b1o3v, "wb3", "l3b0conv2", 16, True,
                    w3prov("l3b0conv2"), residual=sc_3v)
            y2_3 = sm_tile(2, L3)
            y2_3v = y2_3[:].rearrange("p (m s l) -> p m s l", m=2, s=PER)
            conv_s1(psum, tmp, b1o3v, y2_3v, "wb3", "l3b1conv1", 16, True,
                    w3prov("l3b1conv1"))
            l3o = sm_tile(2, L3)
            l3ov = l3o[:].rearrange("p (m s l) -> p m s l", m=2, s=PER)
            conv_s1(psum, tmp, y2_3v, l3ov, "wb3", "l3b1conv2", 16, True,
                    w3prov("l3b1conv2"), residual=b1o3v)

        # -------- layer4: weights streamed in kc-halves (ring bufs=3)
        with ExitStack() as es4:
            l4wp = es4.enter_context(tc.tile_pool(name="l4wp", bufs=3))

            def load_half(pack, name, kcs):
                """Load kc subset of a conv's blocks -> tile + wprov."""
                base0, npos, KC, MC, cin, cout = wmeta(pack, name)
                nk = len(kcs)
                wt = l4wp.tile([128, npos * nk * MC * 128], RES_DT, tag="l4w",
                               name="l4w")
                bv = dr[pack][:].rearrange("p (c x) -> p c x", x=128)
                wv = wt[:].rearrange("p (c x) -> p c x", x=128)
                for i, kc in enumerate(kcs):
                    for mc in range(MC):
                        src = bv[:, base0 // 128 + kc * MC + mc : base0 // 128 + npos * KC * MC : KC * MC, :]
                        dstv = wv[:, i * MC + mc : nk * MC * npos : nk * MC, :]
                        nc.sync.dma_start(dstv, src)

                def g(pos, kc, mc, ks, ms):
                    i = kcs.index(kc)
                    col = ((pos * nk + i) * MC + mc) * 128
                    return wt[0:ks, col : col + ms]
                return g

            def half_wprov(pack, name):
                _, npos, KC, MC, cin, cout = wmeta(pack, name)
                halves = [list(range(KC))[: (KC + 1) // 2],
                          list(range(KC))[(KC + 1) // 2 :]]
                provs = {}
                for kcs in halves:
                    if not kcs:
                        continue
                    p = load_half(pack, name, kcs)
                    for kc in kcs:
                        provs[kc] = p
                def g(pos, kc, mc, ks, ms):
                    return provs[kc](pos, kc, mc, ks, ms)
                return g

            # phase planes of l3out: plane[kc][py][px][img][r, c] = grid[2r+py, 2c+px]
            with ExitStack() as esp:
                plp = esp.enter_context(tc.tile_pool(name="plp", bufs=1))
                pt = plp.tile([128, 2 * 2 * 2 * PER * 80], RES_DT)
                ptv = pt[:].rearrange("p (k y x s f) -> p k y x s f", k=2, y=2,
                                      x=2, s=PER)
                xg3 = l3ov.rearrange("p k s (r c) -> p k s r c", r=16)
                for kc in range(2):
                    for py in range(2):
                        for px in range(2):
                            dstp = ptv[:, kc, py, px, :, 0:64].rearrange(
                                "p s (r c) -> p s r c", r=8)
                            nc.vector.tensor_copy(
                                dstp, xg3[:, kc, :, py : 16 : 2, px : 16 : 2])

                def l4s2(y_v, name, wprov, relu, positions):
                    _, npos, KC, MC, cin, cout = wmeta("wb4a", name)
                    yg = y_v.rearrange("p m s (r c) -> p m s r c", r=9)
                    for mc in range(MC):
                        pc = psum.tile([128, 512], F32, tag="pconv", name="pconv")
                        pcv = pc[:, 0 : PER * 64].rearrange("p (s f) -> p s f", s=PER)
                        first = True
                        for pi, (pos, py, px, sy, sx) in enumerate(positions):
                            for kc in range(KC):
                                off = sy * 8 + sx
                                v = ptv[:, kc, py, px, :, off : off + 64]
                                nc.tensor.matmul(
                                    pcv, wprov(pos, kc, mc, 128, 128), v,
                                    start=first,
                                    stop=(pi == len(positions) - 1 and kc == KC - 1))
                                first = False
                        pg = pcv.rearrange("p s (r c) -> p s r c", r=8)
                        dst = yg[:, mc, :, 1:8, 1:8]
                        nc.scalar.activation(dst, pg[:, :, 0:7, 0:7],
                                             AF.Relu if relu else AF.Identity,
                                             bias=bias_ap(name, mc, 128))

                pos3 = []
                for dy in range(3):
                    py, sy = (dy % 2, dy // 2) if dy != 1 else (1, 0)
                    for dx in range(3):
                        px, sx = (dx % 2, dx // 2) if dx != 1 else (1, 0)
                        pos3.append((dy * 3 + dx, py, px, sy, sx))
                y1_4 = sm_tile(4, L4)
                y1_4v = y1_4[:].rearrange("p (m s l) -> p m s l", m=4, s=PER)
                l4s2(y1_4v, "l4b0conv1", half_wprov("wb4a", "l4b0conv1"), True, pos3)
                halo_zero(y1_4v, 9)
                sc_4 = sm_tile(4, L4)
                sc_4v = sc_4[:].rearrange("p (m s l) -> p m s l", m=4, s=PER)
                l4s2(sc_4v, "l4down", half_wprov("wb4a", "l4down"), False,
                     [(0, 1, 1, 0, 0)])
                halo_zero(sc_4v, 9)
            b1o4 = sm_tile(4, L4)
            b1o4v = b1o4[:].rearrange("p (m s l) -> p m s l", m=4, s=PER)
            conv_s1(psum, tmp, y1_4v, b1o4v, "wb4b", "l4b0conv2", 9, True,
                    half_wprov("wb4b", "l4b0conv2"), residual=sc_4v)
            y2_4 = sm_tile(4, L4)
            y2_4v = y2_4[:].rearrange("p (m s l) -> p m s l", m=4, s=PER)
            conv_s1(psum, tmp, b1o4v, y2_4v, "wb4c", "l4b1conv1", 9, True,
                    half_wprov("wb4c", "l4b1conv1"))
            l4o = sm_tile(4, L4)
            l4ov = l4o[:].rearrange("p (m s l) -> p m s l", m=4, s=PER)
            conv_s1(psum, tmp, y2_4v, l4ov, "wb4d", "l4b1conv2", 9, True,
                    half_wprov("wb4d", "l4b1conv2"), residual=b1o4v)

        # -------- global sum pool (mean folded into fc weights)
        g4 = l4ov.rearrange("p m s (r c) -> p m s r c", r=9)
        for mc in range(4):
            for s in range(PER):
                nc.vector.tensor_reduce(ztv[:, 2 + mc, s : s + 1], g4[:, mc, s, 1:8, 1:8],
                                        mybir.AxisListType.XY, AluOpType.add)


# ---------------------------------------------------------------- entry
_CACHE = {}


def kernel(image, landmarks, edge_index, params):
    in_maps, meta = _host_prep(image, landmarks, edge_index, params)
    if "prog" not in _CACHE:
        _CACHE["prog"] = _build_program(meta)
    nc = _CACHE["prog"]
    res = run_bass_kernel_spmd(nc, in_maps, list(range(NCORES)))
    out = np.empty((B, NCLS), np.float32)
    for c in range(NCORES):
        out[c * PER : (c + 1) * PER] = res.results[c]["out"].T
    _CACHE["last_results"] = res
    _CACHE["last_in_maps"] = in_maps
    return out
